# revision 40
# baseline (speedup 1.0000x reference)
"""Trainium2 Bass kernel for nn_Attend: 2-layer MLP on A and B, then
bidirectional attention (row/col softmax of f_A @ f_B^T, both applied to B).

Sharding: data-parallel over the 32-sequence batch dim across 8 NeuronCores
(4 sequences per core); MLP weights replicated; each core computes its local
e/beta/alpha independently. No collectives.

Per-core dataflow (per sequence b):
  AT = transpose(A_b) via PE-transpose            [d=768, i=1024]
  hT_A = relu(W1^T-matmul: lhsT=W1, rhs=AT)       [h=1024, i=1024]
  fT_A = relu(lhsT=W2, rhs=hT_A)                  [k=1024, i=1024]
  (same for B; B_b natural layout kept for the attention-apply matmuls)
  E[i,j] = lhsT=fT_A, rhs=fT_B   (computed ONCE)
  X = exp(E - m_i)  (per-row-block subsampled max shift), rowsum via accum
  beta  = (X^T-matmuls against B_nat) / rowsum    [row softmax]
  Xs = X * exp(m_i - M)  (M = global max) == exp(E - M): a single global
  shift makes the COLUMN normalization of Xs the exact column softmax, so
  alpha needs no second E matmul and no transpose: lhsT = Xs natural.
  colsum comes free from a ones-column appended to B_nat in the alpha mm.
"""

import os
import numpy as np

NB = 32          # total batch
S = 1024         # sequence length
D = 768          # input dim
H = 1024         # hidden dim
NCORES = 8
CB = NB // NCORES  # sequences per core

# dtype of matmul streams: "f32r" (full speed) or "f32" (4x slower, exact fp32)
MLP_DT = os.environ.get("ATT_MLP_DT", "f32r")
E_DT = os.environ.get("ATT_E_DT", "f32r")
AV_DT = os.environ.get("ATT_AV_DT", "f32r")

_CACHE = {}


def _split_wide_waits(nc, mybir, max_waits=1):
    """Walrus codegen in this image accepts at most one semaphore wait per
    lowered instruction (LDWEIGHTS and CTRL structs have a single wait
    slot). Split excess waits onto preceding same-engine NOPs (engine
    FIFO order preserves semantics)."""
    n = 0
    for f in nc.m.functions:
        for bb in f.blocks:
            il = bb.instructions
            k = 0
            while k < len(il):
                ins = il[k]
                si = ins.sync_info
                if (
                    si is not None
                    and si.on_wait
                    and len(si.on_wait) > max_waits
                ):
                    waits = list(si.on_wait)
                    chunks = [
                        waits[i : i + max_waits]
                        for i in range(0, len(waits), max_waits)
                    ]
                    for chunk in chunks[:-1]:
                        nop = mybir.InstNoOp(
                            name=f"I-waitsplit-{n}", engine=ins.engine
                        )
                        n += 1
                        nop.sync_info = mybir.SyncInfo(on_wait=chunk, on_update=[])
                        il.insert(k, nop)
                        k += 1
                    ins.sync_info = mybir.SyncInfo(
                        on_wait=chunks[-1], on_update=si.on_update
                    )
                k += 1
    return n


def _build_program(split_waits=True):
    import concourse.bass as bass
    import concourse.bass_isa as bass_isa
    import concourse.mybir as mybir
    import concourse.tile as tile
    from concourse.masks import make_identity

    f32 = mybir.dt.float32
    AF = mybir.ActivationFunctionType
    AX = mybir.AxisListType
    DT = {"f32": f32, "f32r": mybir.dt.float32r}
    mlp_dt = DT[MLP_DT]
    e_dt = DT[E_DT]
    av_dt = DT[AV_DT]

    nc = bass.Bass()
    A_d = nc.dram_tensor("A", [CB, S, D], av_dt, kind="ExternalInput")
    B_d = nc.dram_tensor("B", [CB, S, D], av_dt, kind="ExternalInput")
    W1_d = nc.dram_tensor("W1", [D, H], mlp_dt, kind="ExternalInput")
    b1_d = nc.dram_tensor("b1", [H], f32, kind="ExternalInput")
    W2_d = nc.dram_tensor("W2", [H, H], mlp_dt, kind="ExternalInput")
    b2_d = nc.dram_tensor("b2", [H], f32, kind="ExternalInput")
    beta_d = nc.dram_tensor("beta", [CB, S, D], f32, kind="ExternalOutput")
    alpha_d = nc.dram_tensor("alpha", [CB, S, D], f32, kind="ExternalOutput")
    debug = os.environ.get("ATT_DEBUG", "") == "1"
    if debug:
        dbg_nm8 = nc.dram_tensor("dbg_nm8", [CB, 128, 8], f32,
                                 kind="ExternalOutput")
        dbg_negM = nc.dram_tensor("dbg_negM", [CB, 128, 1], f32,
                                  kind="ExternalOutput")
        dbg_g8 = nc.dram_tensor("dbg_g8", [CB, 128, 8], f32,
                                kind="ExternalOutput")
        dbg_x0 = nc.dram_tensor("dbg_x0", [CB, 128, S], mybir.dt.bfloat16,
                                kind="ExternalOutput")
        dbg_bn0 = nc.dram_tensor("dbg_bn0", [CB, 128, D + 8],
                                 mybir.dt.bfloat16, kind="ExternalOutput")
        dbg_cs = nc.dram_tensor("dbg_cs", [CB, 128, 8], f32,
                                kind="ExternalOutput")

    SB = S // 128   # 8 row blocks per sequence
    DB = D // 128   # 6 d blocks
    HB = H // 128   # 8 h blocks
    NCH = S // 512  # 2 matmul N-chunks per 1024
    DP = D + 8      # padded bnat width (col D holds the ones column)

    with tile.TileContext(nc) as tc:
        with (
            tc.tile_pool(name="main", bufs=1) as mp,
            tc.tile_pool(name="ps", bufs=1, space="PSUM") as pp,
        ):
            # --- constants (once) ---
            ident_f = mp.tile([128, 128], f32, tag="misc_idf", bufs=1,
                              name="ident_f")
            make_identity(nc, ident_f)
            ident = mp.tile([128, 128], av_dt, tag="misc_id", bufs=1, name="ident")
            nc.vector.tensor_copy(ident[:], ident_f[:])
            bf16 = mybir.dt.bfloat16
            ident_b = mp.tile([128, 128], bf16, tag="misc_idb", bufs=1,
                              name="ident_b")
            nc.vector.tensor_copy(ident_b[:], ident_f[:])
            b1sb = mp.tile([128, HB], f32, tag="misc_b1", bufs=1, name="b1sb")
            nc.sync.dma_start(out=b1sb[:], in_=b1_d.rearrange("(c p) -> p c", p=128))
            b2sb = mp.tile([128, HB], f32, tag="misc_b2", bufs=1, name="b2sb")
            nc.sync.dma_start(out=b2sb[:], in_=b2_d.rearrange("(c p) -> p c", p=128))
            ones1f = mp.tile([1, 128], f32, tag="misc_ones1", bufs=1,
                             name="ones1f")
            nc.vector.memset(ones1f[:], 1.0)

            def load_transpose(src_ap, nat_tag, nat_bufs, xt_tiles, pfx,
                                   nat_dt, split_first=0, mk_bf16=False,
                                   alt_queue=True):
                """DMA [S, D] natural row-blocks and PE-transpose into
                xt_tiles (DB tiles of [128, S]). The nat tiles are a small
                stream; with mk_bf16, persistent bf16 shadows are kept (the
                attention-apply rhs; col D is set to 1.0 so the alpha mm
                emits the column-sum for free). split_first: split that many
                leading blocks into 4 partition-sliced DMAs (queue spread)."""
                shadows = []
                for ib in range(SB):
                    nat = mp.tile([128, D], nat_dt, tag=nat_tag, bufs=nat_bufs,
                                  name=f"{pfx}nat{ib}")
                    eng = nc.sync if (ib % 2 == 0 or not alt_queue) \
                        else nc.scalar
                    if ib < split_first:
                        for q in range(4):
                            psl = slice(q * 32, (q + 1) * 32)
                            eng.dma_start(
                                out=nat[psl, :],
                                in_=src_ap[ib * 128:(ib + 1) * 128, :][psl, :])
                    else:
                        eng.dma_start(
                            out=nat[:], in_=src_ap[ib * 128:(ib + 1) * 128, :])
                    if mk_bf16:
                        sh = mp.tile([128, DP], bf16, tag="bb16", bufs=8,
                                     name=f"{pfx}b16_{ib}")
                        nc.vector.tensor_copy(sh[:, :D], nat[:])
                        nc.vector.memset(sh[:, D:D + 1], 1.0)
                        shadows.append(sh)
                    for kd in range(DB):
                        tp = pp.tile([128, 128], nat_dt, tag="tp", bufs=2,
                                     name=f"{pfx}tp{ib}_{kd}")
                        nc.tensor.transpose(
                            tp[:], nat[:, kd * 128:(kd + 1) * 128], ident[:])
                        nc.vector.tensor_copy(
                            xt_tiles[kd][:, ib * 128:(ib + 1) * 128], tp[:])
                return shadows

            def mlp_layer(w_tiles, n_k, x_tiles, out_tag, bias_sb, out_dt,
                              pfx):
                """out[HB tiles of [128,S]] = relu(lhsT=w, rhs=x) + bias."""
                outs = []
                for hb in range(HB):
                    acc = pp.tile([128, S], f32, tag="acc", bufs=3,
                                  name=f"{pfx}acc{hb}")
                    for n in range(NCH):
                        nsl = slice(n * 512, (n + 1) * 512)
                        for kd in range(n_k):
                            nc.tensor.matmul(
                                acc[:, nsl],
                                lhsT=w_tiles[kd][:, hb * 128:(hb + 1) * 128],
                                rhs=x_tiles[kd][:, nsl],
                                start=(kd == 0),
                                stop=(kd == n_k - 1),
                            )
                    o = mp.tile([128, S], out_dt, tag=out_tag, bufs=8,
                                name=f"{pfx}o{hb}")
                    nc.scalar.activation(
                        o[:], acc[:], AF.Relu,
                        bias=bias_sb[:, hb:hb + 1], scale=1.0)
                    outs.append(o)
                return outs

            def load_w(dram, n_k, tag, pfx, eng=None):
                ws = []
                for k in range(n_k):
                    t = mp.tile([128, H], mlp_dt, tag=tag, bufs=8,
                                name=f"{pfx}w{k}")
                    (eng or nc.sync).dma_start(
                        out=t[:], in_=dram[k * 128:(k + 1) * 128, :])
                    ws.append(t)
                return ws

            # A(0) first so the PE can start transposing ASAP; W1 before
            # W2 (L1 needs it first). W2 persistent across batches; W1
            # re-loaded per batch into slots shared with f_BT (tag w1fbt).
            at_next = [mp.tile([128, S], mlp_dt, tag="xt", bufs=6,
                               name=f"at0_{k}") for k in range(DB)]
            load_transpose(A_d[0], "nat", 3, at_next, "a0_", av_dt,
                           split_first=3, alt_queue=False)
            w1 = load_w(W1_d, DB, "w1fbt", "b0_w1_")
            w2 = load_w(W2_d, HB, "w2", "w2_")

            for b in range(CB):
                pfx = f"b{b}_"
                at = at_next

                # --- MLP A ---
                hat = mlp_layer(w1, DB, at, "hTpp", b1sb, mlp_dt, pfx + "h1a")
                fat = mlp_layer(w2, HB, hat, "fat", b2sb, e_dt, pfx + "h2a")

                # --- B side (natural tiles persist for attention) ---
                bt = [mp.tile([128, S], mlp_dt, tag="xt", bufs=6,
                              name=f"{pfx}bt{k}") for k in range(DB)]
                bnat = load_transpose(B_d[b], "nat", 3, bt, pfx + "b",
                                      av_dt, mk_bf16=True)
                hbt = mlp_layer(w1, DB, bt, "hTpp", b1sb, mlp_dt, pfx + "h1b")
                fbt = mlp_layer(w2, HB, hbt, "w1fbt", b2sb, e_dt, pfx + "h2b")

                # --- prefetch next batch's A load+transpose (fills the PE
                # while attention DMAs drain; xt slots of BT free here) ---
                if b + 1 < CB:
                    at_next = [mp.tile([128, S], mlp_dt, tag="xt", bufs=6,
                                       name=f"b{b+1}_at{k}") for k in range(DB)]
                    load_transpose(A_d[b + 1], "nat", 3, at_next,
                                   f"a{b+1}_", av_dt)

                # per-row-block -max values land in nm8 columns; the
                # early-global shift M6 (blocks 0..6) is reduced from it
                # while block 7's E matmul still runs.
                nm8 = mp.tile([128, SB], f32, tag="nm8", bufs=1,
                              name=f"{pfx}nm8")

                # --- attention phase 1: all 8 E row-blocks, exp'd with
                # per-row subsampled-max shifts into X (bf16). The ACT exps
                # pipeline one block behind the PE's E matmuls. ---
                def emit_e_mms(ib):
                    isl = slice(ib * 128, (ib + 1) * 128)
                    acc = pp.tile([128, S], f32, tag="acc", bufs=3,
                                  name=f"{pfx}e{ib}")
                    for n in range(NCH):
                        nsl = slice(n * 512, (n + 1) * 512)
                        for kk in range(HB):
                            nc.tensor.matmul(
                                acc[:, nsl],
                                lhsT=fat[kk][:, isl],
                                rhs=fbt[kk][:, nsl],
                                start=(kk == 0),
                                stop=(kk == HB - 1),
                            )
                    return acc

                def emit_softmax(ib, acc):
                    # A strided-subsample max is a valid softmax shift (the
                    # shift cancels; exp of the small positive residual
                    # cannot overflow) and is 8x cheaper on DVE.
                    sub = acc.rearrange("p (a b) -> p a b", b=8)[:, :, 0]
                    nc.vector.reduce_max(nm8[:, ib:ib + 1], sub, axis=AX.X,
                                         negate=True)
                    sblk = mp.tile([128, S], bf16, tag="sblk", bufs=8,
                                   name=f"{pfx}s{ib}")
                    nc.scalar.activation(
                        sblk[:], acc[:], AF.Exp,
                        bias=nm8[:, ib:ib + 1], scale=1.0)
                    return sblk

                xblocks = []
                for ib in range(SB):
                    acc = emit_e_mms(ib)
                    xblocks.append(emit_softmax(ib, acc))
                    if ib == 5:
                        # Early global shift: M = max over blocks 0..5 of
                        # the per-row maxes (within a few sigma of the true
                        # max; exp of the residual stays far inside bf16
                        # range). mrow is ready while block 6's E matmul
                        # streams, so the PE transpose below never stalls.
                        mrow = mp.tile([128, 1], f32, tag="stats", bufs=16,
                                       name=f"{pfx}mrow")
                        # nm8 holds -m; min over blocks negated = max_b m.
                        nc.vector.tensor_reduce(
                            mrow[:], nm8[:, 0:6],
                            op=mybir.AluOpType.min, axis=AX.X, negate=True)
                    elif ib == 6:
                        # tiny PE transpose right after e-mm(6): data ready.
                        tpm = pp.tile([1, 128], f32, tag="tp", bufs=2,
                                      name=f"{pfx}tpm")
                        nc.tensor.transpose(tpm[:], mrow[:], ident_f[:])
                        negMv = mp.tile([1, 1], f32, tag="stats1", bufs=2,
                                        name=f"{pfx}negMv")
                        nc.vector.reduce_max(negMv[:], tpm[:], axis=AX.X,
                                             negate=True)

                # broadcast -M to all partitions with a 1-row matmul
                # (lands right after e-mm(7) on the PE), then g = exp(m-M)
                # and the per-block Xs row scales.
                negMp = pp.tile([128, 1], f32, tag="tp", bufs=2,
                                name=f"{pfx}negMp")
                nc.tensor.matmul(negMp[:], lhsT=ones1f[:], rhs=negMv[:],
                                 start=True, stop=True)
                negM = mp.tile([128, 1], f32, tag="stats", bufs=16,
                               name=f"{pfx}negM")
                nc.vector.tensor_copy(negM[:], negMp[:])
                g8 = mp.tile([128, SB], f32, tag="g8", bufs=2,
                             name=f"{pfx}g8")
                nc.scalar.activation(g8[:], nm8[:], AF.Exp,
                                     bias=negM[:], scale=-1.0)
                # g never touches X: it rides as the CS matmul's stationary
                # vector (bf16) and as the alpha output-stage row scale.
                gb = mp.tile([128, SB], bf16, tag="g8b", bufs=2,
                             name=f"{pfx}gb")
                nc.vector.tensor_copy(gb[:], g8[:])

                # --- attention phase 2: shared-transpose beta+alpha.
                # Xs == exp(E - M6) up to bf16 rounding. Per output block i:
                #   lhsT tiles Xs^T(jb, ib) from 8 PE transposes of xs[ib].
                #   beta = (Xs^T-mms vs bnat+ones) / rowsum (ones column).
                #   alpha = (Xs^T scaled per-partition j by 1/colsum_j) @ B
                # (the reference's alpha softmax normalizes over i within
                # each column j, then applies transposed: the 1/CS_j factor
                # rides in the second evacuation of the same transposes).
                def emit_cs():
                    # CS[j] = sum_i g_i X(i, j): g-row matmuls over the 8
                    # natural X tiles; [1, S] PSUM row -> DMA scatter to
                    # [128, SB] per-partition layout -> reciprocal.
                    csacc = pp.tile([1, S], f32, tag="acc", bufs=3,
                                    name=f"{pfx}csacc")
                    for n in range(NCH):
                        nsl = slice(n * 512, (n + 1) * 512)
                        for r in range(SB):
                            nc.tensor.matmul(
                                csacc[:, nsl],
                                lhsT=gb[:, r:r + 1],
                                rhs=xblocks[r][:, nsl],
                                start=(r == 0),
                                stop=(r == SB - 1),
                            )
                    # DVE evac permutes CS into (row-within-block major)
                    # order so the scatter DMA is a plain reshape.
                    csrow = mp.tile([1, S], f32, tag="nmflat", bufs=1,
                                    name=f"{pfx}csrow")
                    nc.scalar.activation(
                        csrow[:].rearrange("p (q c) -> p q c", c=SB),
                        csacc.rearrange("p (c q) -> p q c", q=128),
                        AF.Copy, bias=0.0, scale=1.0)
                    csT = mp.tile([128, SB], f32, tag="cst", bufs=2,
                                  name=f"{pfx}csT")
                    nc.gpsimd.dma_start(
                        out=csT[:],
                        in_=csrow[:].rearrange("p (a b) -> p a b", b=SB))
                    rcs = mp.tile([128, SB], f32, tag="cst", bufs=2,
                                  name=f"{pfx}rcs")
                    nc.vector.reciprocal(rcs[:], csT[:])
                    # bs[j] = B[j,:] / CS_j: the alpha matmul rhs. Lives in
                    # the fat-tag slots (dead after the E matmuls).
                    bs = []
                    for jb in range(SB):
                        t = mp.tile([128, DP], bf16, tag="fat", bufs=8,
                                    name=f"{pfx}bs{jb}")
                        nc.vector.tensor_scalar_mul(
                            t[:, :D], bnat[jb][:, :D], rcs[:, jb:jb + 1])
                        bs.append(t)
                    return rcs, bs

                def emit_beta_block(ib, _unused=None):
                    isl = slice(ib * 128, (ib + 1) * 128)
                    # 4 transposes share one PSUM tile (disjoint 128-col
                    # chunks); evacuated twice: plain (beta lhsT) and
                    # scaled by 1/CS_j per partition (alpha lhsT).
                    sts, sus = [], []
                    for g in range(2):
                        tp4 = pp.tile([128, 512], bf16, tag="tp", bufs=2,
                                      name=f"{pfx}stp{ib}_{g}")
                        for j4 in range(4):
                            jb = g * 4 + j4
                            nc.tensor.transpose(
                                tp4[:, j4 * 128:(j4 + 1) * 128],
                                xblocks[ib][:, jb * 128:(jb + 1) * 128],
                                ident_b[:])
                        st4 = mp.tile([128, 512], bf16, tag="st", bufs=4,
                                      name=f"{pfx}st{ib}_{g}")
                        nc.vector.tensor_copy(st4[:], tp4[:])
                        sts.append(st4)
                        sus.append(st4)  # placeholder; su evac in alpha
                    oacc = pp.tile([128, S], f32, tag="acc", bufs=3,
                                   name=f"{pfx}betaacc{ib}")
                    for jb in range(SB):
                        lhs = sts[jb // 4][:, (jb % 4) * 128:(jb % 4 + 1) * 128]
                        for csl in (slice(0, 512), slice(512, D + 1)):
                            nc.tensor.matmul(
                                oacc[:, csl],
                                lhsT=lhs,
                                rhs=bnat[jb][:, csl],
                                start=(jb == 0),
                                stop=(jb == SB - 1),
                            )
                    recip = mp.tile([128, 1], f32, tag="stats", bufs=16,
                                    name=f"{pfx}rb{ib}")
                    nc.vector.reciprocal(recip[:], oacc[:, D:D + 1])
                    stage = mp.tile([128, D], f32, tag="ostage", bufs=2,
                                    name=f"{pfx}bstage{ib}")
                    nc.scalar.activation(
                        stage[:], oacc[:, :D], AF.Copy, bias=0.0,
                        scale=recip[:])
                    nc.sync.dma_start(out=beta_d[b, isl, :], in_=stage[:])
                    return sts

                def emit_alpha_block(ib, sts):
                    isl = slice(ib * 128, (ib + 1) * 128)
                    oacc2 = pp.tile([128, S], f32, tag="acc", bufs=3,
                                    name=f"{pfx}alphaacc{ib}")
                    for jb in range(SB):
                        lhs = sts[jb // 4][:, (jb % 4) * 128:(jb % 4 + 1) * 128]
                        for csl in (slice(0, 512), slice(512, D)):
                            nc.tensor.matmul(
                                oacc2[:, csl],
                                lhsT=lhs,
                                rhs=bs[jb][:, csl],
                                start=(jb == 0),
                                stop=(jb == SB - 1),
                            )
                    stage2 = mp.tile([128, D], f32, tag="ostage", bufs=2,
                                     name=f"{pfx}astage{ib}")
                    nc.scalar.activation(
                        stage2[:], oacc2[:, :D], AF.Copy, bias=0.0,
                        scale=g8[:, ib:ib + 1])
                    nc.sync.dma_start(out=alpha_d[b, isl, :], in_=stage2[:])

                # PE order: CS (first matmul only needs xs[0], the last
                # lands just after scale(7)) | beta(0) | beta(1) | alpha(0)
                # | beta(2) | alpha(1) | ... -- su bufs=2 carries block k's
                # scaled transposes to its alpha one step later.
                sts0 = emit_beta_block(0, None)
                rcs, bs = emit_cs()
                if b + 1 < CB:
                    w1 = load_w(W1_d, DB, "w1fbt", f"b{b+1}_w1_")
                sus_prev = sts0
                for ib in range(1, SB):
                    sus_cur = emit_beta_block(ib)
                    emit_alpha_block(ib - 1, sus_prev)
                    sus_prev = sus_cur
                emit_alpha_block(SB - 1, sus_prev)

    if split_waits:
        _split_wide_waits(nc, mybir)
    return nc


def _get_program():
    if "nc" not in _CACHE:
        _CACHE["nc"] = _build_program()
    return _CACHE["nc"]


def _run(A, B, W1, b1, W2, b2, **spmd_kwargs):
    from concourse.bass_utils import run_bass_kernel_spmd

    nc = _get_program()
    in_maps = []
    for c in range(NCORES):
        sl = slice(c * CB, (c + 1) * CB)
        in_maps.append({
            "A": np.ascontiguousarray(A[sl], dtype=np.float32),
            "B": np.ascontiguousarray(B[sl], dtype=np.float32),
            "W1": np.asarray(W1, dtype=np.float32),
            "b1": np.asarray(b1, dtype=np.float32),
            "W2": np.asarray(W2, dtype=np.float32),
            "b2": np.asarray(b2, dtype=np.float32),
        })
    return run_bass_kernel_spmd(nc, in_maps, list(range(NCORES)), **spmd_kwargs)


def kernel(A, B, W1, b1, W2, b2):
    res = _run(A, B, W1, b1, W2, b2)
    beta = np.concatenate([res.results[c]["beta"] for c in range(NCORES)], axis=0)
    alpha = np.concatenate([res.results[c]["alpha"] for c in range(NCORES)], axis=0)
    return beta, alpha
